# revision 1
# baseline (speedup 1.0000x reference)
"""Trainium2 Bass kernel for the autoregressive GRU decode head.

Problem: context = mean over zones of encoded_features[:, -1]  -> (B, D)
then 12 autoregressive steps of a 2-layer GRU (H=256) + linear projection
to N=256 zones.  B=1024, data-parallel across 8 NeuronCores (128 batch each).

Structure (per core, feature-major / "transposed" activations):
  actT (128p, 4 slots, 128) bf16 : [h0 c0, h0 c1, h1 c0, h1 c1]
       slot holds h[b, c*128 + p] at [p, b]    (c = chunk of the 256-dim)
  Gate tensors (PSUM) use layout [p, c*128 + b].
  Matmuls: out(gate_chunk, B) = lhsT.T @ rhs, lhsT = W^T tile (K<=128, M=128),
  rhs = actT slot (K=128, B=128), K-chunks accumulated in PSUM.
  The prediction feedback is algebraically folded into layer 0's weights:
  W_pred @ (W_out h1 + b_out) = (W_pred W_out) h1 + W_pred b_out.
  Chain per layer (PSUM g_rz = [r|z], g_hi = [ghn|gin]):
    r,z = sigmoid(g_rz + bias)   4x 128-wide ACT (bias via ACT operand)
    t = (ghn + bhn) * r          STT 256w
    v = (gin + bin) + t          STT 256w
    n = tanh(v); c = z*h; q = (z-1)*n; h' = c - q
  Critical-path trick: the NEXT matmul block needs W @ h' with h' = c - q,
  so it is issued as W @ c + (-W) @ q against the c/q tiles, which exist
  ~1us before h' does -- the r-gate matmuls of the next layer/step complete
  almost immediately after the chain, instead of serializing behind h'.
  The z / in-gate matmuls use h' directly (they are needed later).
  All h1(t-1)-only work runs during chain0; all h0'(t)-only work (incl.
  step t+1's rz0/hn0 h-parts) during chain1; wout(t-1) during chain0.
The encoded_features slice is streamed as bf16 (host-converted) in 6 chunks
(64,64,64,32,16,16 zones) ALL on the sync HWDGE queue so arrival order
matches the tree's processing order; weights/biases ride the scalar queue.
The zone mean is a DVE-only pairwise TT-add tree (bf16 upper levels, f32
tail) overlapped with the DMA.  GpSimd is avoided: it shares an SBUF port
with the DVE and large concurrent ops slow both ~4x.
"""

import sys

for _p in ("/opt/trn_rl_repo",):
    if _p not in sys.path:
        sys.path.insert(0, _p)

import numpy as np
import ml_dtypes

import concourse.bass as bass
import concourse.tile as tile
from concourse import mybir
from concourse.vector_clock import ScopedClock

BF16 = ml_dtypes.bfloat16

B, T, NZ, D = 1024, 8, 256, 256
H = 256
STEPS = 12
N_CORES = 8
PC = B // N_CORES  # 128 batch per core

F32 = mybir.dt.float32
BF = mybir.dt.bfloat16
F8 = mybir.dt.float8e4
AF = mybir.ActivationFunctionType
OP = mybir.AluOpType

# bias_sb (f32, [128, NCOL]): per-partition bias columns;
# value[p, col] = bias of gate index c*128+p of the owning chunk
_RZ0 = 0                      # 12 steps * 4 chunks (r c0, r c1, z c0, z c1)
_RZ1 = _RZ0 + STEPS * 4       # 4
_HN0 = _RZ1 + 4               # 2
_HN1 = _HN0 + 2               # 2
_IN0 = _HN1 + 2               # 12 steps * 2
_IN1 = _IN0 + STEPS * 2       # 2
NCOL = _IN1 + 2

# brow (bf16, [1, 2*128]): b_out as a moving row for the batch-major wout
_BOUT = 0
NROW = 2


def _install_tile_drain_patch():
    """walrus (CoreV3) rejects >1 sync wait on the tail drain; spill extras
    onto preceding sync nops."""
    if getattr(tile.TileContext, "_drain_patch_installed", False):
        return

    def _patched(self, tick_clock, wait_clock):
        nc = self.nc
        bb = nc.cur_bb.bb
        drain_bi = nc.sync.drain()
        drain_inst = drain_bi.ins
        wait_clock.add_sem_waits(
            drain_inst, ScopedClock({None: tick_clock.global_clock})
        )
        w = drain_inst.sync_info.on_wait if drain_inst.sync_info else None
        maxw = 1
        if w and len(w) > maxw:
            extra = list(w[maxw:])
            drain_inst.sync_info.on_wait = list(w[:maxw])
            idx = bb.instructions.index(drain_inst)
            nops = []
            for i in range(0, len(extra), maxw):
                nop_bi = nc.sync.nop()
                nop = nop_bi.ins
                si = nop.sync_info
                nop.sync_info = mybir.SyncInfo(
                    on_wait=extra[i : i + maxw],
                    on_update=(si.on_update if si else []),
                )
                bb.instructions.remove(nop)
                nops.append(nop)
            bb.instructions[idx:idx] = nops
        nc.all_engine_barrier()
        popped = nc._tile_sem_poison_stack.pop()
        assert popped is self._sem_poison
        nc.clear_and_free_semaphores(list(self.sems.allocated().values()))
        nc.all_engine_barrier()

    tile.TileContext._drain_and_barrier = _patched
    tile.TileContext._drain_patch_installed = True


def _split_waits(nc, maxw=1):
    """This walrus build rejects instructions carrying more than ~1 sem
    wait; spill extra waits onto same-engine nops placed just before."""
    for bb in nc.main_func.blocks:
        new_list = []
        for inst in bb.instructions:
            si = inst.sync_info
            w = list(si.on_wait) if si and si.on_wait else []
            if len(w) > maxw:
                keep = w[len(w) - maxw:]
                extra = w[: len(w) - maxw]
                si.on_wait = keep
                for i in range(0, len(extra), maxw):
                    nop = mybir.InstNoOp(
                        name=f"{inst.name}-sw{i}", ins=[], outs=[]
                    )
                    nop.engine = inst.engine
                    nop.sync_info = mybir.SyncInfo(
                        on_wait=extra[i : i + maxw], on_update=[]
                    )
                    nc.register_instruction(nop)
                    new_list.append(nop)
            new_list.append(inst)
        bb.instructions[:] = new_list


class _Group:
    """Tracks start/stop flags for a PSUM accumulation group whose matmuls
    are emitted in several program-order batches."""

    def __init__(self, total):
        self.total = total
        self.emitted = 0

    def flags(self):
        start = self.emitted == 0
        self.emitted += 1
        return start, self.emitted == self.total


def build_kernel(nsteps=12):
    """Build the per-core Bass graph (SPMD: same graph on all 8 cores)."""
    _install_tile_drain_patch()
    nc = bass.Bass()

    enc = nc.declare_dram_parameter("enc", [PC, NZ, D], F8, isOutput=False)
    wrz0 = nc.declare_dram_parameter("wrz0", [4, 128, 512], BF, isOutput=False)
    wrz0n = nc.declare_dram_parameter("wrz0n", [2, 128, 512], BF,
                                      isOutput=False)
    win0 = nc.declare_dram_parameter("win0", [2, 128, 256], BF, isOutput=False)
    whn0 = nc.declare_dram_parameter("whn0", [2, 128, 256], BF, isOutput=False)
    wrz1 = nc.declare_dram_parameter("wrz1", [4, 128, 512], BF, isOutput=False)
    wrz1n = nc.declare_dram_parameter("wrz1n", [2, 128, 512], BF,
                                      isOutput=False)
    win1 = nc.declare_dram_parameter("win1", [2, 128, 256], BF, isOutput=False)
    whn1 = nc.declare_dram_parameter("whn1", [2, 128, 256], BF, isOutput=False)
    wout = nc.declare_dram_parameter("wout", [2, 128, 256], BF, isOutput=False)
    biases = nc.declare_dram_parameter("biases", [128, NCOL], F32,
                                       isOutput=False)
    brows = nc.declare_dram_parameter("brows", [1, NROW * 128], BF,
                                      isOutput=False)
    out = nc.declare_dram_parameter("out", [PC, STEPS, NZ], BF, isOutput=True)

    with tile.TileContext(nc) as tc:
        with (
            tc.tile_pool(name="consts", bufs=1) as consts,
            tc.tile_pool(name="state", bufs=1) as state,
            tc.tile_pool(name="enc_pool", bufs=4) as enc_pool,
            tc.tile_pool(name="gates", bufs=2) as gates,
            tc.tile_pool(name="ostage", bufs=2) as ostage,
            tc.tile_pool(name="psum", bufs=1, space="PSUM") as psum,
        ):
            # ---- phase 1 DMA: enc all on the sync queue, in tree order ----
            ZCHS = [32, 32, 32, 32, 32, 32, 32, 16, 16]
            PE_CHUNKS = (3, 4, 5, 6, 7)  # zone-summed on the idle PE instead
            NCH = len(ZCHS)
            e_tiles = []
            z0 = 0
            for i, zch in enumerate(ZCHS):
                e_sb = enc_pool.tile([128, 32 * D], F8, tag="echunk", bufs=9)
                nc.sync.dma_start(e_sb[:, : zch * D], enc[:, z0 : z0 + zch, :])
                e_tiles.append(e_sb)
                z0 += zch

            # weights / biases on the scalar queue
            w_sb = {}
            for name, ap, kc, mdim in (
                ("wrz0", wrz0, 4, 512),
                ("whn0", whn0, 2, 256),
                ("wrz1", wrz1, 4, 512),
                ("whn1", whn1, 2, 256),
                ("wrz0n", wrz0n, 2, 512),
                ("wrz1n", wrz1n, 2, 512),
                ("win0", win0, 2, 256),
                ("win1", win1, 2, 256),
                ("wout", wout, 2, 256),
            ):
                t_ = consts.tile([128, kc, mdim], BF, tag=name)
                nc.scalar.dma_start(t_[:], ap.rearrange("k p m -> p k m"))
                w_sb[name] = t_
            bias_sb = consts.tile([128, NCOL], F32, tag="bias")
            nc.scalar.dma_start(bias_sb[:], biases[:])
            brow_sb = consts.tile([1, NROW * 128], BF, tag="brow")
            nc.scalar.dma_start(brow_sb[:], brows[:])

            ones_row = consts.tile([1, 128], BF, tag="ones")
            nc.gpsimd.memset(ones_row[:], 1.0)
            identity = consts.tile([128, 128], F32, tag="ident")
            nc.gpsimd.memset(identity[:], 0.0)
            nc.gpsimd.affine_select(
                out=identity[:],
                in_=identity[:],
                compare_op=OP.not_equal,
                fill=1.0,
                base=0,
                pattern=[[-1, 128]],
                channel_multiplier=1,
            )
            ident_f8 = consts.tile([128, 128], F8, tag="identf8")
            nc.gpsimd.memset(ident_f8[:], 0.0)
            nc.gpsimd.affine_select(
                out=ident_f8[:],
                in_=ident_f8[:],
                compare_op=OP.not_equal,
                fill=1.0,
                base=0,
                pattern=[[-1, 128]],
                channel_multiplier=1,
            )
            # prewarm the sigmoid/tanh ACT table during phase 1
            warm = consts.tile([128, 1], F32, tag="warm")
            nc.scalar.activation(warm[:], identity[:, 0:1], AF.Sigmoid)
            gwarm = consts.tile([128, 128], BF, tag="gwarm")
            nc.gpsimd.tensor_tensor(gwarm[:], ident_f8[:], ident_f8[:],
                                    OP.add)

            # ---- phase 1: zone-mean; DVE pairwise tree for most chunks,
            # PE identity-matmul accumulation for PE_CHUNKS (the PE is
            # otherwise idle during the stream; PSUM accumulates in f32)
            tmpf = state.tile([128, 512], F32, tag="tmpf")
            ptl = state.tile([128, 256], F32, tag="ptl")
            acc = state.tile([128, 256], F32, tag="acc")
            zsum = psum.tile([128, 256], F32, tag="outp", bufs=2)
            n_pe = sum(ZCHS[i] for i in PE_CHUNKS)
            pe_grp = _Group(n_pe)
            first_dve = True
            for i in range(NCH):
                e_sb = e_tiles[i]
                if i in PE_CHUNKS:
                    for z in range(ZCHS[i]):
                        st, sp = pe_grp.flags()
                        nc.tensor.matmul(
                            zsum[:], ident_f8[:],
                            e_sb[:, z * D : (z + 1) * D],
                            start=st, stop=sp,
                        )
                    continue
                w = ZCHS[i] * D
                scr = state.tile([128, 16 * D], BF, tag="scr")
                h = w // 2
                nc.vector.tensor_tensor(
                    scr[:, 0:h], e_sb[:, 0:h], e_sb[:, h:w], OP.add
                )
                w = h
                while w > 4 * D:
                    h = w // 2
                    nc.vector.tensor_tensor(
                        scr[:, 0:h], scr[:, 0:h], scr[:, h:w], OP.add
                    )
                    w = h
                nc.vector.tensor_tensor(
                    tmpf[:], scr[:, 0 : 2 * D], scr[:, 2 * D : 4 * D], OP.add
                )
                if first_dve:
                    nc.vector.tensor_tensor(
                        acc[:], tmpf[:, 0:D], tmpf[:, D : 2 * D], OP.add
                    )
                    first_dve = False
                else:
                    nc.vector.tensor_tensor(
                        ptl[:], tmpf[:, 0:D], tmpf[:, D : 2 * D], OP.add
                    )
                    nc.vector.tensor_tensor(acc[:], acc[:], ptl[:], OP.add)
            ztot = state.tile([128, 256], F32, tag="ztot")
            nc.scalar.activation(ztot[:], zsum[:], AF.Copy)

            # ---- state: actT slots [h0c0, h0c1, h1c0, h1c1] ----
            # the acc + ztot merge rides the PSUM accumulation of the
            # two transposes
            actT = state.tile([128, 4, 128], BF, tag="actT")
            for c in range(2):
                cs = slice(c * 128, (c + 1) * 128)
                ctps = psum.tile([128, 128], F32, tag="outp", bufs=2)
                nc.tensor.matmul(ctps[:], acc[:, cs], identity[:],
                                 is_transpose=True, start=True, stop=False)
                nc.tensor.matmul(ctps[:], ztot[:, cs], identity[:],
                                 is_transpose=True, start=False, stop=True)
                nc.scalar.activation(actT[:, c, :], ctps[:], AF.Copy,
                                     scale=1.0 / NZ)
                nc.scalar.activation(actT[:, 2 + c, :], ctps[:], AF.Copy,
                                     scale=1.0 / NZ)

            # ---- decode-phase emitters ----
            def gate_mms(g, grp, w_t, kis, slots, mlo, mhi, coloff=0):
                """slots entries: int -> actT slot; (tile, k) -> gates tile
                chunk k used as the moving operand."""
                for m in range(mlo, mhi):
                    ms = slice((coloff + m) * 128, (coloff + m + 1) * 128)
                    wms = slice(m * 128, (m + 1) * 128)
                    for ki, slot in zip(kis, slots):
                        if isinstance(slot, tuple):
                            src, k = slot
                            rhs = src[:, k * 128 : (k + 1) * 128]
                        else:
                            rhs = actT[:, slot, :]
                        st, sp = grp.flags()
                        nc.tensor.matmul(
                            g[:, ms], w_t[:, ki, wms], rhs, start=st, stop=sp,
                        )

            # ---- phase 2: 12 decode steps ----
            cur = {}

            def emit_pre0(t):
                """rz0-hh / hn0 for step t: depend only on h0(t-1)."""
                g_rz0 = psum.tile([128, 512], F32, tag="rz0", bufs=2)
                g_hi0 = psum.tile([128, 512], F32, tag="hi0", bufs=2)
                grz0 = _Group(8 + (12 if t > 0 else 0))
                ghn0 = _Group(4)
                gin0 = _Group(4 if t > 0 else 0)
                gate_mms(g_rz0, grz0, w_sb["wrz0"], (2, 3), (0, 1), 0, 4)
                gate_mms(g_hi0, ghn0, w_sb["whn0"], (0, 1), (0, 1), 0, 2)
                cur[t] = (g_rz0, g_hi0, grz0, ghn0, gin0)

            emit_pre0(0)
            prev_w = None
            prev_cq = None   # (c_, q_) of the most recent layer-1 chain

            def emit_wout(t):
                g_pb = psum.tile([128, 256], F32, tag="outp", bufs=2)
                gout = _Group(3)
                st, sp = gout.flags()
                nc.tensor.matmul(
                    g_pb[:], ones_row[:],
                    brow_sb[0:1, _BOUT * 128 : (_BOUT + 2) * 128],
                    start=st, stop=sp,
                )
                for ki, slot in ((0, 2), (1, 3)):
                    st, sp = gout.flags()
                    nc.tensor.matmul(
                        g_pb[:], actT[:, slot, :], w_sb["wout"][:, ki, :],
                        start=st, stop=sp,
                    )
                o_ = ostage.tile([128, 256], BF, tag="ost")
                nc.scalar.activation(o_[:], g_pb[:], AF.Copy)
                nc.sync.dma_start(out[:, t, :], o_[:])

            for t in range(nsteps):
                g_rz0, g_hi0, grz0, ghn0, gin0 = cur.pop(t)
                g_rz1 = psum.tile([128, 512], F32, tag="rz1", bufs=1)
                g_hi1 = psum.tile([128, 512], F32, tag="hi1", bufs=1)
                grz1 = _Group(8 + 12)
                ghn1 = _Group(4)
                gin1 = _Group(4)

                for layer in range(2):
                    if layer == 0:
                        g_rz, g_hi, grz, gin = g_rz0, g_hi0, grz0, gin0
                        rz_col = _RZ0 + t * 4
                        hn_col, in_col = _HN0, _IN0 + t * 2
                        h_sl, x_sl = 0, (2, 3)      # h slots; x = other h
                        w_f, w_fn = w_sb["wrz0"], w_sb["wrz0n"]
                        w_in = w_sb["win0"]
                        cq = prev_cq if t > 0 else None
                    else:
                        g_rz, g_hi, grz, gin = g_rz1, g_hi1, grz1, gin1
                        rz_col = _RZ1
                        hn_col, in_col = _HN1, _IN1
                        h_sl, x_sl = 2, (0, 1)
                        w_f, w_fn = w_sb["wrz1"], w_sb["wrz1n"]
                        w_in = w_sb["win1"]
                        cq = this_cq  # layer-0 chain of this step

                    # r-part of the input block via W@c + (-W)@q: fires as
                    # soon as the previous chain's c/q exist (before h')
                    s_ = gates.tile([128, 512], BF, tag=f"s{layer}")
                    if cq is not None:
                        c_p, q_p = cq
                        gate_mms(g_rz, grz, w_f, (0, 1),
                                 ((c_p, 0), (c_p, 1)), 0, 2)
                        gate_mms(g_rz, grz, w_fn, (0, 1),
                                 ((q_p, 0), (q_p, 1)), 0, 2)
                    for c in (0, 1):
                        nc.scalar.activation(
                            s_[:, c * 128 : (c + 1) * 128],
                            g_rz[:, c * 128 : (c + 1) * 128], AF.Sigmoid,
                            bias=bias_sb[:, rz_col + c : rz_col + c + 1],
                        )
                    # z-part plainly on h' of the previous chain
                    if cq is not None:
                        gate_mms(g_rz, grz, w_f, (0, 1), x_sl, 2, 4)
                    # z c0 now; z c1 is deferred until after tanh c0 so the
                    # ACT engine serves n_c0 as early as possible
                    nc.scalar.activation(
                        s_[:, 256:384], g_rz[:, 256:384], AF.Sigmoid,
                        bias=bias_sb[:, rz_col + 2 : rz_col + 3],
                    )
                    # n path
                    t_ = gates.tile([128, 256], BF, tag=f"t{layer}")
                    for c in range(2):
                        cs = slice(c * 128, (c + 1) * 128)
                        nc.vector.scalar_tensor_tensor(
                            t_[:, cs], g_hi[:, cs],
                            bias_sb[:, hn_col + c : hn_col + c + 1],
                            s_[:, cs], op0=OP.add, op1=OP.mult,
                        )
                    # in-gate mms on h' (emitted after t_ so t_'s PSUM dep
                    # closes at the pre-emitted hn mms)
                    if cq is not None:
                        gate_mms(g_hi, gin, w_in, (0, 1), x_sl, 0, 2,
                                 coloff=2)
                    if layer == 0:
                        # layer-1 parts that need only h1(t-1)
                        gate_mms(g_rz1, grz1, w_sb["wrz1"], (2, 3), (2, 3),
                                 0, 4)
                        gate_mms(g_hi1, ghn1, w_sb["whn1"], (0, 1), (2, 3),
                                 0, 2)
                    else:
                        # step t+1 parts that need only h0'(t)
                        if t + 1 < nsteps:
                            emit_pre0(t + 1)
                    v_ = gates.tile([128, 256], BF, tag=f"v{layer}")
                    for c in range(2):
                        cs = slice(c * 128, (c + 1) * 128)
                        bcol = bias_sb[:, in_col + c : in_col + c + 1]
                        if cq is not None or layer == 1:
                            nc.vector.scalar_tensor_tensor(
                                v_[:, cs], g_hi[:, 256 + c * 128 :
                                                256 + (c + 1) * 128],
                                bcol, t_[:, cs], op0=OP.add, op1=OP.add,
                            )
                        else:
                            nc.vector.tensor_scalar_add(
                                v_[:, cs], t_[:, cs], bcol,
                            )
                    # chunked tail: n/c/q/h' per 128-chunk so chunk 0 of
                    # c/q (the next matmul block's operands) lands early
                    c_ = gates.tile([128, 256], BF, tag=f"c{layer}")
                    n_ = gates.tile([128, 256], BF, tag=f"n{layer}")
                    q_ = gates.tile([128, 256], BF, tag=f"q{layer}")
                    for c in range(2):
                        cs = slice(c * 128, (c + 1) * 128)
                        zs = slice(256 + c * 128, 256 + (c + 1) * 128)
                        nc.scalar.activation(n_[:, cs], v_[:, cs], AF.Tanh)
                        if c == 0:
                            nc.scalar.activation(
                                s_[:, 384:512], g_rz[:, 384:512], AF.Sigmoid,
                                bias=bias_sb[:, rz_col + 3 : rz_col + 4],
                            )
                        nc.gpsimd.tensor_tensor(
                            c_[:, cs], s_[:, zs], actT[:, h_sl + c, :],
                            OP.mult,
                        )
                        nc.vector.scalar_tensor_tensor(
                            q_[:, cs], s_[:, zs], 1.0, n_[:, cs],
                            op0=OP.subtract, op1=OP.mult,
                        )
                        nc.vector.tensor_tensor(
                            actT[:, h_sl + c, :], c_[:, cs], q_[:, cs],
                            OP.subtract,
                        )

                    if layer == 0:
                        this_cq = (c_, q_)
                        if prev_w is not None:
                            emit_wout(prev_w)
                    else:
                        prev_cq = (c_, q_)
                prev_w = t
            emit_wout(prev_w)

    _split_waits(nc)
    return nc


def _prep_inputs(encoded_features, step_emb, W_ih0, W_hh0, b_ih0, b_hh0,
                 W_ih1, W_hh1, b_ih1, b_hh1, W_out, b_out):
    """Host-side: slice/shard the big input, transpose + cast weights,
    fold the output projection into layer-0 input weights, fold the
    step-embedding matmul + all additive constants into bias columns."""
    f4 = np.float32
    enc_last = np.asarray(encoded_features)[:, -1].astype(ml_dtypes.float8_e4m3)
    enc_last = np.ascontiguousarray(enc_last)

    W_ih0 = np.asarray(W_ih0, f4)
    W_hh0 = np.asarray(W_hh0, f4)
    W_ih1 = np.asarray(W_ih1, f4)
    W_hh1 = np.asarray(W_hh1, f4)
    W_out = np.asarray(W_out, f4)
    step_emb = np.asarray(step_emb, f4)
    b_ih0 = np.asarray(b_ih0, f4)
    b_hh0 = np.asarray(b_hh0, f4)
    b_ih1 = np.asarray(b_ih1, f4)
    b_hh1 = np.asarray(b_hh1, f4)
    b_out = np.asarray(b_out, f4)

    W_emb = W_ih0[:, :D]          # (768, 256)
    W_pred = W_ih0[:, D:]         # (768, 256)
    W_fold = W_pred @ W_out       # (768, 256): pred feedback folded onto h1
    b_fold = W_pred @ b_out       # (768,)

    # gi_emb[t] = W_emb @ step_emb[t] + b_ih0  -> (12, 768)
    gi_emb = step_emb[:STEPS] @ W_emb.T + b_ih0[None, :]

    def kstack(*mats_cols):
        chunks = []
        for mat, cols in mats_cols:
            mt = np.ascontiguousarray(mat.T[:, cols])  # (K, M)
            for k in range(0, mt.shape[0], 128):
                chunks.append(mt[k : k + 128])
        return np.stack(chunks).astype(BF16)  # (nk, 128, M)

    rz = slice(0, 512)
    ng = slice(512, 768)
    wrz0 = kstack((W_fold, rz), (W_hh0, rz))          # K: h1c0,h1c1,h0c0,h0c1
    wrz0n = kstack((-W_fold, rz))
    win0 = kstack((W_fold, ng))
    whn0 = kstack((W_hh0, ng))
    wrz1 = kstack((W_ih1, rz), (W_hh1, rz))           # K: h0c0,h0c1,h1c0,h1c1
    wrz1n = kstack((-W_ih1, rz))
    win1 = kstack((W_ih1, ng))
    whn1 = kstack((W_hh1, ng))
    wout = np.stack([np.ascontiguousarray(W_out.T)[k : k + 128] for k in (0, 128)]
                    ).astype(BF16)                    # (2, 128, 256)

    biases = np.zeros((128, NCOL), f4)

    def putc(base, vec):
        for c in range(len(vec) // 128):
            biases[:, base + c] = vec[c * 128 : (c + 1) * 128]

    for t in range(STEPS):
        extra = b_fold if t > 0 else np.zeros_like(b_fold)
        putc(_RZ0 + t * 4, gi_emb[t, :512] + b_hh0[:512] + extra[:512])
        putc(_IN0 + t * 2, gi_emb[t, 512:] + extra[512:])
    putc(_RZ1, b_ih1[:512] + b_hh1[:512])
    putc(_HN0, b_hh0[512:])
    putc(_HN1, b_hh1[512:])
    putc(_IN1, b_ih1[512:])

    brows = np.zeros(NROW * 128, f4)
    brows[_BOUT * 128 : _BOUT * 128 + 256] = b_out
    brows = brows.astype(BF16)[None, :]

    shared = dict(wrz0=wrz0, wrz0n=wrz0n, win0=win0, whn0=whn0, wrz1=wrz1,
                  wrz1n=wrz1n, win1=win1, whn1=whn1, wout=wout,
                  biases=biases, brows=brows)
    in_maps = []
    for i in range(N_CORES):
        m = dict(shared)
        m["enc"] = enc_last[i * PC : (i + 1) * PC]
        in_maps.append(m)
    return in_maps


_CACHE = {}


def _run(in_maps, trace=False):
    from concourse.bass_utils import run_bass_kernel_spmd

    if "nc" not in _CACHE:
        _CACHE["nc"] = build_kernel()
    nc = _CACHE["nc"]
    res = run_bass_kernel_spmd(
        nc, in_maps, core_ids=list(range(N_CORES)), trace=trace
    )
    preds = np.concatenate([res.results[i]["out"] for i in range(N_CORES)],
                       axis=0).astype(np.float32)
    return preds, res


def kernel(encoded_features, step_emb, W_ih0, W_hh0, b_ih0, b_hh0,
           W_ih1, W_hh1, b_ih1, b_hh1, W_out, b_out, num_steps):
    assert int(num_steps) == STEPS
    in_maps = _prep_inputs(encoded_features, step_emb, W_ih0, W_hh0, b_ih0,
                           b_hh0, W_ih1, W_hh1, b_ih1, b_hh1, W_out, b_out)
    preds, _ = _run(in_maps, trace=False)
    return preds

